# revision 17
# baseline (speedup 1.0000x reference)
"""Trainium2 Bass kernel for nn_MultiHeadAttention (BS=4, L=2048, D=1024, H=16).

Sharding: 8 cores = 4 batches x 2 query-halves. Each core computes attention
for 1024 query rows of one batch over all 16 heads, with K/V restricted to
that batch's unmasked key positions (host-side gather; masked keys contribute
exactly 0 to the reference softmax, and padding slots get bias -1e30 so
exp() makes them exactly 0 on device too). No cross-core communication; the
host concatenates the 8 [1024, 1024] outputs.

Per-core dataflow (everything kept transposed / d-major so no on-device
transposes are ever needed):
  A3: V[tok, dv]   = (xkv^T)^T @ Wv + bv      (stationary = xkv^T tiles)
      stored with a ones-column appended per head (65 cols/head) so that the
      PV matmul also produces the softmax denominator in its 65th row.
  A2: K^T[dv, tok] = Wk^T-tiles @ xkv^T + bk
  A1: Q^T[dv, tok] = Wq^T-tiles @ xq^T + bq
  B:  per (query-chunk qc of 512, head-pair p):
        S^T[k,q] via row-packed K=64 matmul pairs (2 heads concurrently on
        the PE via partition offsets 0/64) -> 2-bank PSUM [128, 1024]
        exp via one ScalarE ACTIVATE (scale=1/8, bias = mask column) -> P^T
        O^T[0:64] + sums[64] accumulate via M=65 PV matmuls; sums rows are
        staged 32-aligned, inverted with 4 lane-parallel exact reciprocals
        per chunk, broadcast across partitions with 0-stride DMAs from a
        DRAM bounce, and multiplied in-place into o_t.
  C:  out[q, n] = o_t-tiles^T @ Wf + bf, streamed out per 128-row tile.

Matmul operands are bf16 with N=1024 moving where PSUM allows (fp32r lowers
to the 2x-slower fp32-HIGH path on this toolchain); accumulation is fp32 in
PSUM. Emission order interleaves the projection tail and fc with the
ACT-bound attention phase so the Tile scheduler can fill PE gaps.
"""
import sys

sys.path.insert(0, "/opt/trn_rl_repo")

import numpy as np
import ml_dtypes

BF16NP = ml_dtypes.bfloat16

BS, L, D, H, DK = 4, 2048, 1024, 16, 64
SCALE = 1.0 / np.sqrt(DK)
QH = L // 2          # queries per core
NP = H // 2          # head pairs
MT = D // 128        # out-dim tiles
DT = D // 128        # contraction d tiles
NEG = -1.0e30

_programs = {}
_ONES = np.ones((128, 128), BF16NP)


def _chunks(total, maxc=512):
    n = -(-total // maxc)
    base = total // n
    rem = total - base * n
    return [base + (1 if i < rem else 0) for i in range(n)]


def _build(KC):
    import contextlib

    import concourse.bacc as bacc
    import concourse.bass as bass
    import concourse.tile as tile
    import concourse.mybir as mybir

    F32 = mybir.dt.float32
    BF16 = mybir.dt.bfloat16
    AF = mybir.ActivationFunctionType
    ts = bass.ts

    KCT = KC // 128
    kch = _chunks(KC)

    nc = bacc.Bacc("TRN2", target_bir_lowering=False)

    xq_d = nc.dram_tensor("xq", [D, QH], BF16, kind="ExternalInput")
    xkv_d = nc.dram_tensor("xkv", [D, KC], BF16, kind="ExternalInput")
    wq_d = nc.dram_tensor("wq", [D, D], BF16, kind="ExternalInput")
    wk_d = nc.dram_tensor("wk", [D, D], BF16, kind="ExternalInput")
    wv_d = nc.dram_tensor("wv", [D, D], BF16, kind="ExternalInput")
    wf_d = nc.dram_tensor("wf", [D, D], BF16, kind="ExternalInput")
    mb_d = nc.dram_tensor("mb", [128, KCT], F32, kind="ExternalInput")
    ones_d = nc.dram_tensor("ones", [128, 128], BF16, kind="ExternalInput")
    bq_d = nc.dram_tensor("bqt", [128, MT], F32, kind="ExternalInput")
    bk_d = nc.dram_tensor("bkt", [128, MT], F32, kind="ExternalInput")
    bv_d = nc.dram_tensor("bvr", [1, D], BF16, kind="ExternalInput")
    bf_d = nc.dram_tensor("bfr", [1, D], BF16, kind="ExternalInput")
    out_d = nc.dram_tensor("out", [QH, D], F32, kind="ExternalOutput")

    with tile.TileContext(nc) as tc, nc.allow_low_precision(
        reason="bf16 matmul pipeline with fp32 accumulation"
    ), contextlib.ExitStack() as ctx:
        const = ctx.enter_context(tc.tile_pool(name="const", bufs=1))
        persist = ctx.enter_context(tc.tile_pool(name="persist", bufs=1))
        work = ctx.enter_context(tc.tile_pool(name="work", bufs=1))
        ppool = ctx.enter_context(tc.tile_pool(name="ppool", bufs=3))
        rcpool = ctx.enter_context(tc.tile_pool(name="rcpool", bufs=1))
        bcpool = ctx.enter_context(tc.tile_pool(name="bcpool", bufs=2))
        outpool = ctx.enter_context(tc.tile_pool(name="outpool", bufs=1))
        psum = ctx.enter_context(tc.tile_pool(name="psum", bufs=3, space="PSUM"))
        drpool = ctx.enter_context(tc.tile_pool(name="drpool", bufs=1, space="DRAM"))

        ones128 = const.tile([128, 128], BF16, name="ones128")
        nc.sync.dma_start(ones128[:], ones_d[:])
        mb_sb = const.tile([128, KCT], F32, name="mb_sb")
        nc.sync.dma_start(mb_sb[:], mb_d[:])
        bq_sb = const.tile([128, MT], F32, name="bq_sb")
        nc.sync.dma_start(bq_sb[:], bq_d[:])
        bk_sb = const.tile([128, MT], F32, name="bk_sb")
        nc.sync.dma_start(bk_sb[:], bk_d[:])
        bv_sb = const.tile([1, D], BF16, name="bv_sb")
        nc.sync.dma_start(bv_sb[:], bv_d[:])
        bf_sb = const.tile([1, D], BF16, name="bf_sb")
        nc.sync.dma_start(bf_sb[:], bf_d[:])

        q_t = [persist.tile([128, QH], BF16, name=f"q_t{m}", tag=f"q_t{m}")
               for m in range(MT)]
        k_t = [persist.tile([128, KC], BF16, name=f"k_t{m}", tag=f"k_t{m}")
               for m in range(MT)]
        v65 = [persist.tile([128, H * 65], BF16, name=f"v65_{t}", tag=f"v65_{t}")
               for t in range(KCT)]
        o_t = [persist.tile([128, QH], BF16, name=f"o_t{p}", tag=f"o_t{p}")
               for p in range(NP)]
        xkv = [persist.tile([128, KC], BF16, name=f"xkv{d}", tag=f"xkv{d}")
               for d in range(DT)]
        xq = [persist.tile([128, QH], BF16, name=f"xq{d}", tag=f"xq{d}")
              for d in range(DT)]
        wv = [persist.tile([128, D], BF16, name=f"wv{d}", tag=f"wv{d}")
              for d in range(DT)]
        wk = [persist.tile([128, D], BF16, name=f"wk{d}", tag=f"wk{d}")
              for d in range(DT)]
        wq = [persist.tile([128, D], BF16, name=f"wq{d}", tag=f"wq{d}")
              for d in range(DT)]
        wf = [persist.tile([128, D], BF16, name=f"wf{d}", tag=f"wf{d}")
              for d in range(DT)]
        stg = [work.tile([128, 512], F32, name=f"stg{g}", tag=f"stg{g}")
               for g in range(4)]
        for g in range(4):
            nc.vector.memset(stg[g][:], 1.0)
        rdr = [drpool.tile([H, 512], F32, name=f"rdr{qc}", tag=f"rdr{qc}")
               for qc in range(2)]

        for d in range(DT):
            nc.sync.dma_start(xkv[d][:], xkv_d[ts(d, 128), :])
            nc.sync.dma_start(wv[d][:], wv_d[ts(d, 128), :])
        for t in range(KCT):
            v3 = v65[t].rearrange("p (h e) -> p h e", e=65)
            nc.vector.tensor_copy(
                v3[:, :, 64:65],
                ones128[:, 0:H].rearrange("p (h e) -> p h e", e=1))

        # ---------------- phase pieces ----------------
        def a3(trange):
            # V (k-major) with ones column interleaved; N=1024 moving
            for t in trange:
                pv = psum.tile([128, 1024], F32, name=f"pv{t}", tag="ps1024")
                for d in range(DT):
                    for c in range(2):
                        nc.tensor.matmul(pv[:, ts(c, 512)],
                                         xkv[d][:, ts(t, 128)],
                                         wv[d][:, ts(c, 512)],
                                         start=(d == 0), stop=False)
                for c in range(2):
                    nc.tensor.matmul(pv[:, ts(c, 512)], ones128[0:1, 0:128],
                                     bv_sb[0:1, ts(c, 512)],
                                     start=False, stop=True)
                dst = v65[t].rearrange("p (h e) -> p h e", e=65)
                src = pv.rearrange("p (h e) -> p h e", e=64)
                nc.vector.tensor_copy(dst[:, :, 0:64], src[:])

        def a2(mrange):
            for m in mrange:
                pk = [psum.tile([128, w], F32, name=f"pk{m}_{c}", tag="ps1024")
                      for c, w in enumerate(kch)]
                for d in range(DT):
                    off = 0
                    for c, w in enumerate(kch):
                        nc.tensor.matmul(
                            pk[c][:], wk[d][:, ts(m, 128)],
                            xkv[d][:, off:off + w],
                            start=(d == 0), stop=(d == DT - 1))
                        off += w
                off = 0
                for c, w in enumerate(kch):
                    nc.vector.tensor_scalar_add(
                        k_t[m][:, off:off + w], pk[c][:], bk_sb[:, m:m + 1])
                    off += w

        def a1(mrange):
            for m in mrange:
                pq = psum.tile([128, 1024], F32, name=f"pq{m}", tag="ps1024")
                for d in range(DT):
                    for c in range(2):
                        nc.tensor.matmul(pq[:, ts(c, 512)],
                                         wq[d][:, ts(m, 128)],
                                         xq[d][:, ts(c, 512)],
                                         start=(d == 0), stop=(d == DT - 1))
                nc.vector.tensor_scalar_add(q_t[m][:], pq[:], bq_sb[:, m:m + 1])

        def b_pairs(qc, prange):
            for p in prange:
                h0, h1 = 2 * p, 2 * p + 1
                o0 = psum.tile([128, 512], F32, name=f"o0_{qc}_{p}", tag="oB",
                               bufs=2)
                o1 = psum.tile([128, 512], F32, name=f"o1_{qc}_{p}", tag="oB",
                               bufs=2)
                for kt in range(KCT):
                    s = psum.tile([128, 1024], F32,
                                  name=f"s_{qc}_{p}_{kt}", tag="ps1024")
                    nc.tensor.matmul(
                        s[:, 0:512], k_t[p][0:64, ts(kt, 128)],
                        q_t[p][0:64, ts(qc, 512)])
                    nc.tensor.matmul(
                        s[:, 512:1024], k_t[p][64:128, ts(kt, 128)],
                        q_t[p][64:128, ts(qc, 512)])
                    pp = ppool.tile([128, 1024], BF16,
                                    name=f"pp_{qc}_{p}_{kt}", tag="pp")
                    nc.scalar.activation(
                        pp[:], s[:], AF.Exp,
                        bias=mb_sb[:, kt:kt + 1], scale=float(SCALE))
                    nc.tensor.matmul(
                        o0[0:65, :], v65[kt][:, h0 * 65:(h0 + 1) * 65],
                        pp[:, 0:512],
                        start=(kt == 0), stop=(kt == KCT - 1))
                    nc.tensor.matmul(
                        o1[0:65, :], v65[kt][:, h1 * 65:(h1 + 1) * 65],
                        pp[:, 512:1024],
                        start=(kt == 0), stop=(kt == KCT - 1))
                nc.vector.tensor_copy(o_t[p][0:64, ts(qc, 512)], o0[0:64, :])
                nc.vector.tensor_copy(o_t[p][64:128, ts(qc, 512)], o1[0:64, :])
                # stage sums rows (head h -> tile h%4, row 32*(h//4);
                # engine partition starts must be 32-aligned)
                for h, ops in ((h0, o0), (h1, o1)):
                    g, r = h % 4, 32 * (h // 4)
                    nc.vector.tensor_copy(stg[g][r:r + 1, :], ops[64:65, :])

        def b_denoms(qc):
            # exact reciprocals, 4 lanes-parallel [128,512] calls, then
            # DRAM bounce + 0-stride broadcast + in-place normalize
            rstg = [rcpool.tile([128, 512], F32,
                                name=f"rstg_{qc}_{g}", tag=f"rstg{g}")
                    for g in range(4)]
            for g in range(4):
                nc.vector.reciprocal(rstg[g][:], stg[g][:])
                for hh in range(4):
                    h = hh * 4 + g
                    nc.sync.dma_start(
                        rdr[qc][h:h + 1, :], rstg[g][32 * hh:32 * hh + 1, :])
            for p in range(NP):
                bcs = bcpool.tile([128, 1024], F32,
                                  name=f"bcs_{qc}_{p}", tag="bcs")
                nc.sync.dma_start(
                    bcs[0:64, 0:512],
                    rdr[qc][2 * p:2 * p + 1, :].to_broadcast([64, 512]))
                nc.sync.dma_start(
                    bcs[64:128, 512:1024],
                    rdr[qc][2 * p + 1:2 * p + 2, :].to_broadcast([64, 512]))
                nc.vector.tensor_mul(
                    o_t[p][0:64, ts(qc, 512)],
                    o_t[p][0:64, ts(qc, 512)], bcs[0:64, 0:512])
                nc.vector.tensor_mul(
                    o_t[p][64:128, ts(qc, 512)],
                    o_t[p][64:128, ts(qc, 512)], bcs[64:128, 512:1024])

        def c_fc(qtrange):
            for qt in qtrange:
                fp = psum.tile([128, 1024], F32, name=f"fp{qt}", tag="ps1024")
                for dt in range(DT):
                    for c in range(2):
                        nc.tensor.matmul(fp[:, ts(c, 512)],
                                         o_t[dt][:, ts(qt, 128)],
                                         wf[dt][:, ts(c, 512)],
                                         start=(dt == 0), stop=False)
                for c in range(2):
                    nc.tensor.matmul(fp[:, ts(c, 512)], ones128[0:1, 0:128],
                                     bf_sb[0:1, ts(c, 512)],
                                     start=False, stop=True)
                ost = outpool.tile([128, 1024], F32, name=f"ost{qt}", tag="ost")
                nc.vector.tensor_copy(ost[:], fp[:])
                nc.sync.dma_start(out_d[ts(qt, 128), :], ost[:])

        # ---------------- emission order ----------------
        a3(range(KCT))
        for d in range(DT):
            nc.sync.dma_start(wk[d][:], wk_d[ts(d, 128), :])
        a2(range(0, 4))
        for d in range(DT):
            nc.sync.dma_start(xq[d][:], xq_d[ts(d, 128), :])
            nc.sync.dma_start(wq[d][:], wq_d[ts(d, 128), :])
        a1(range(0, 4))
        b_pairs(0, range(0, 4))
        a2(range(4, 8))
        a1(range(4, 8))
        b_pairs(0, range(4, 8))
        b_denoms(0)
        for d in range(DT):
            nc.sync.dma_start(wf[d][:], wf_d[ts(d, 128), :])
        b_pairs(1, range(0, 2))
        c_fc(range(0, 2))
        b_pairs(1, range(2, 4))
        c_fc(range(2, 4))
        b_pairs(1, range(4, 8))
        b_denoms(1)
        c_fc(range(4, 8))

    nc.finalize()
    return nc


_LDW_PATCHED = False


def _enable_ldw_opt():
    global _LDW_PATCHED
    if _LDW_PATCHED:
        return
    import concourse.bass_utils as bu
    orig = bu.run_command

    def patched(cmd, *a, **k):
        cmd = [c.replace("--enable-ldw-opt=false", "--enable-ldw-opt=true")
               if isinstance(c, str) else c for c in cmd]
        return orig(cmd, *a, **k)

    bu.run_command = patched
    _LDW_PATCHED = True


def _get_program(KC):
    if KC not in _programs:
        if __import__("os").environ.get("LDW_OPT"):
            _enable_ldw_opt()
        _programs[KC] = _build(KC)
    return _programs[KC]


LAST_EXEC_NS = None
PROFILE = False


def _ensure_profile_hook():
    """Wire up the NTFF profile hook that the slim agent container leaves
    unconnected (antenv.axon_hooks is not injected; the ctypes hook body
    ships in trn_agent_boot)."""
    import types

    try:
        from antenv.axon_hooks import get_axon_ntff_profile_hook  # noqa: F401
        return
    except ImportError:
        pass
    import antenv

    mod = types.ModuleType("antenv.axon_hooks")
    _h = [None]
    mod.set_axon_ntff_profile_hook = lambda h: _h.__setitem__(0, h)
    mod.get_axon_ntff_profile_hook = lambda: _h[0]
    sys.modules["antenv.axon_hooks"] = mod
    antenv.axon_hooks = mod
    from trn_agent_boot.trn_boot import _ntff_profile_via_ctypes

    mod.set_axon_ntff_profile_hook(
        _ntff_profile_via_ctypes("/opt/axon/libaxon_pjrt.so"))
    # artifact upload needs a bucket this container doesn't have
    import concourse.bass_utils as bu

    bu.upload_artifacts = lambda tmpdir: f"local:{tmpdir}"


def kernel(x, mask, Wq, bq, Wk, bk, Wv, bv, Wf, bf):
    global LAST_EXEC_NS
    from concourse.bass_utils import run_bass_kernel_spmd

    if PROFILE:
        _ensure_profile_hook()

    x = np.asarray(x, dtype=np.float32)
    mask = np.asarray(mask)
    Wq16, Wk16, Wv16, Wf16 = (
        np.ascontiguousarray(np.asarray(w).astype(BF16NP))
        for w in (Wq, Wk, Wv, Wf))
    bq, bk = (np.asarray(v, np.float32) for v in (bq, bk))
    bv16, bf16v = (np.asarray(v).astype(BF16NP).reshape(1, D)
                   for v in (bv, bf))

    keeps = [np.nonzero(np.asarray(mask[b]) == 0)[0] for b in range(BS)]
    maxk = max(1, max(len(k) for k in keeps))
    KC = -(-maxk // 128) * 128
    nc = _get_program(KC)
    KCT = KC // 128

    bq_t = np.ascontiguousarray(bq.reshape(MT, 128).T)
    bk_t = np.ascontiguousarray(bk.reshape(MT, 128).T)

    x16 = x.astype(BF16NP)
    in_maps = []
    for c in range(8):
        b, j = divmod(c, 2)
        keep = keeps[b]
        xq_t = np.ascontiguousarray(x16[b, j * QH:(j + 1) * QH, :].T)
        xkv_t = np.zeros((D, KC), BF16NP)
        xkv_t[:, :len(keep)] = x16[b, keep, :].T
        mbv = np.full(KC, NEG, np.float32)
        mbv[:len(keep)] = 0.0
        mb_t = np.ascontiguousarray(mbv.reshape(KCT, 128).T)
        in_maps.append({
            "ones": _ONES, "xq": xq_t, "xkv": xkv_t,
            "wq": Wq16, "wk": Wk16, "wv": Wv16, "wf": Wf16,
            "mb": mb_t, "bqt": bq_t, "bkt": bk_t,
            "bvr": bv16, "bfr": bf16v,
        })

    res = run_bass_kernel_spmd(nc, in_maps, core_ids=list(range(8)),
                               trace=PROFILE)
    if res.exec_time_ns is not None:
        LAST_EXEC_NS = res.exec_time_ns

    out = np.empty((BS, L, D), np.float32)
    for c in range(8):
        b, j = divmod(c, 2)
        out[b, j * QH:(j + 1) * QH, :] = res.results[c]["out"]
    return out


# revision 19
# speedup vs baseline: 1.0016x; 1.0016x over previous
"""Trainium2 Bass kernel for nn_MultiHeadAttention (BS=4, L=2048, D=1024, H=16).

Sharding: 8 cores = 4 batches x 2 query-halves. Each core computes attention
for 1024 query rows of one batch over all 16 heads, with K/V restricted to
that batch's unmasked key positions (host-side gather; masked keys contribute
exactly 0 to the reference softmax, and padding slots get bias -1e30 so
exp() makes them exactly 0 on device too). No cross-core communication; the
host concatenates the 8 [1024, 1024] outputs.

Per-core dataflow (everything kept transposed / d-major so no on-device
transposes are ever needed):
  A3: V[tok, dv]   = (xkv^T)^T @ Wv + bv      (stationary = xkv^T tiles)
      stored with a ones-column appended per head (65 cols/head) so that the
      PV matmul also produces the softmax denominator in its 65th row.
  A2: K^T[dv, tok] = Wk^T-tiles @ xkv^T + bk
  A1: Q^T[dv, tok] = Wq^T-tiles @ xq^T + bq
  B:  per (query-chunk qc of 512, head-pair p):
        S^T[k,q] via row-packed K=64 matmul pairs (2 heads concurrently on
        the PE via partition offsets 0/64) -> 2-bank PSUM [128, 1024]
        exp via one ScalarE ACTIVATE (scale=1/8, bias = mask column) -> P^T
        O^T[0:64] + sums[64] accumulate via M=65 PV matmuls; sums rows are
        staged 32-aligned, inverted with 4 lane-parallel exact reciprocals
        per chunk, broadcast across partitions with 0-stride DMAs from a
        DRAM bounce, and multiplied in-place into o_t.
  C:  out[q, n] = o_t-tiles^T @ Wf + bf, streamed out per 128-row tile.

Matmul operands are bf16 with N=1024 moving where PSUM allows (fp32r lowers
to the 2x-slower fp32-HIGH path on this toolchain); accumulation is fp32 in
PSUM. Emission order interleaves the projection tail and fc with the
ACT-bound attention phase so the Tile scheduler can fill PE gaps.
"""
import sys

sys.path.insert(0, "/opt/trn_rl_repo")

import numpy as np
import ml_dtypes

BF16NP = ml_dtypes.bfloat16

BS, L, D, H, DK = 4, 2048, 1024, 16, 64
SCALE = 1.0 / np.sqrt(DK)
QH = L // 2          # queries per core
NP = H // 2          # head pairs
MT = D // 128        # out-dim tiles
DT = D // 128        # contraction d tiles
NEG = -1.0e30

_programs = {}
_ONES = np.ones((128, 128), BF16NP)


def _chunks(total, maxc=512):
    n = -(-total // maxc)
    base = total // n
    rem = total - base * n
    return [base + (1 if i < rem else 0) for i in range(n)]


def _build(KC):
    import contextlib

    import concourse.bacc as bacc
    import concourse.bass as bass
    import concourse.tile as tile
    import concourse.mybir as mybir

    F32 = mybir.dt.float32
    BF16 = mybir.dt.bfloat16
    AF = mybir.ActivationFunctionType
    ts = bass.ts

    KCT = KC // 128
    kch = _chunks(KC)

    nc = bacc.Bacc("TRN2", target_bir_lowering=False)

    xq_d = nc.dram_tensor("xq", [D, QH], BF16, kind="ExternalInput")
    xkv_d = nc.dram_tensor("xkv", [D, KC], BF16, kind="ExternalInput")
    wq_d = nc.dram_tensor("wq", [D, D], BF16, kind="ExternalInput")
    wk_d = nc.dram_tensor("wk", [D, D], BF16, kind="ExternalInput")
    wv_d = nc.dram_tensor("wv", [D, D], BF16, kind="ExternalInput")
    wf_d = nc.dram_tensor("wf", [D, D], BF16, kind="ExternalInput")
    mb_d = nc.dram_tensor("mb", [128, KCT], F32, kind="ExternalInput")
    ones_d = nc.dram_tensor("ones", [128, 128], BF16, kind="ExternalInput")
    bq_d = nc.dram_tensor("bqt", [128, MT], F32, kind="ExternalInput")
    bk_d = nc.dram_tensor("bkt", [128, MT], F32, kind="ExternalInput")
    bv_d = nc.dram_tensor("bvr", [1, D], BF16, kind="ExternalInput")
    bf_d = nc.dram_tensor("bfr", [1, D], BF16, kind="ExternalInput")
    out_d = nc.dram_tensor("out", [QH, D], F32, kind="ExternalOutput")

    with tile.TileContext(nc) as tc, nc.allow_low_precision(
        reason="bf16 matmul pipeline with fp32 accumulation"
    ), contextlib.ExitStack() as ctx:
        const = ctx.enter_context(tc.tile_pool(name="const", bufs=1))
        persist = ctx.enter_context(tc.tile_pool(name="persist", bufs=1))
        work = ctx.enter_context(tc.tile_pool(name="work", bufs=1))
        ppool = ctx.enter_context(tc.tile_pool(name="ppool", bufs=3))
        rcpool = ctx.enter_context(tc.tile_pool(name="rcpool", bufs=1))
        bcpool = ctx.enter_context(tc.tile_pool(name="bcpool", bufs=2))
        outpool = ctx.enter_context(tc.tile_pool(name="outpool", bufs=1))
        psum = ctx.enter_context(tc.tile_pool(name="psum", bufs=3, space="PSUM"))
        drpool = ctx.enter_context(tc.tile_pool(name="drpool", bufs=1, space="DRAM"))

        ones128 = const.tile([128, 128], BF16, name="ones128")
        nc.sync.dma_start(ones128[:], ones_d[:])
        mb_sb = const.tile([128, KCT], F32, name="mb_sb")
        nc.sync.dma_start(mb_sb[:], mb_d[:])
        bq_sb = const.tile([128, MT], F32, name="bq_sb")
        nc.sync.dma_start(bq_sb[:], bq_d[:])
        bk_sb = const.tile([128, MT], F32, name="bk_sb")
        nc.sync.dma_start(bk_sb[:], bk_d[:])
        bv_sb = const.tile([1, D], BF16, name="bv_sb")
        nc.sync.dma_start(bv_sb[:], bv_d[:])
        bf_sb = const.tile([1, D], BF16, name="bf_sb")
        nc.sync.dma_start(bf_sb[:], bf_d[:])

        q_t = [persist.tile([128, QH], BF16, name=f"q_t{m}", tag=f"q_t{m}")
               for m in range(MT)]
        k_t = [persist.tile([128, KC], BF16, name=f"k_t{m}", tag=f"k_t{m}")
               for m in range(MT)]
        v65 = [persist.tile([128, H * 65], BF16, name=f"v65_{t}", tag=f"v65_{t}")
               for t in range(KCT)]
        o_t = [persist.tile([128, QH], BF16, name=f"o_t{p}", tag=f"o_t{p}")
               for p in range(NP)]
        xkv = [persist.tile([128, KC], BF16, name=f"xkv{d}", tag=f"xkv{d}")
               for d in range(DT)]
        xq = [persist.tile([128, QH], BF16, name=f"xq{d}", tag=f"xq{d}")
              for d in range(DT)]
        wv = [persist.tile([128, D], BF16, name=f"wv{d}", tag=f"wv{d}")
              for d in range(DT)]
        wk = [persist.tile([128, D], BF16, name=f"wk{d}", tag=f"wk{d}")
              for d in range(DT)]
        wq = [persist.tile([128, D], BF16, name=f"wq{d}", tag=f"wq{d}")
              for d in range(DT)]
        wf = [persist.tile([128, D], BF16, name=f"wf{d}", tag=f"wf{d}")
              for d in range(DT)]
        stg = [work.tile([128, 512], F32, name=f"stg{g}", tag=f"stg{g}")
               for g in range(4)]
        for g in range(4):
            nc.vector.memset(stg[g][:], 1.0)
        rdr = [drpool.tile([H, 512], F32, name=f"rdr{qc}", tag=f"rdr{qc}")
               for qc in range(2)]

        for d in range(DT):
            nc.sync.dma_start(xkv[d][:], xkv_d[ts(d, 128), :])
            nc.sync.dma_start(wv[d][:], wv_d[ts(d, 128), :])
        for t in range(KCT):
            v3 = v65[t].rearrange("p (h e) -> p h e", e=65)
            nc.vector.tensor_copy(
                v3[:, :, 64:65],
                ones128[:, 0:H].rearrange("p (h e) -> p h e", e=1))

        # ---------------- phase pieces ----------------
        def a3(trange):
            # V (k-major) with ones column interleaved; N=1024 moving
            for t in trange:
                pv = psum.tile([128, 1024], F32, name=f"pv{t}", tag="ps1024")
                for d in range(DT):
                    for c in range(2):
                        nc.tensor.matmul(pv[:, ts(c, 512)],
                                         xkv[d][:, ts(t, 128)],
                                         wv[d][:, ts(c, 512)],
                                         start=(d == 0), stop=False)
                for c in range(2):
                    nc.tensor.matmul(pv[:, ts(c, 512)], ones128[0:1, 0:128],
                                     bv_sb[0:1, ts(c, 512)],
                                     start=False, stop=True)
                dst = v65[t].rearrange("p (h e) -> p h e", e=65)
                src = pv.rearrange("p (h e) -> p h e", e=64)
                nc.vector.tensor_copy(dst[:, :, 0:64], src[:])

        def a2(mrange):
            for m in mrange:
                pk = [psum.tile([128, w], F32, name=f"pk{m}_{c}", tag="ps1024")
                      for c, w in enumerate(kch)]
                for d in range(DT):
                    off = 0
                    for c, w in enumerate(kch):
                        nc.tensor.matmul(
                            pk[c][:], wk[d][:, ts(m, 128)],
                            xkv[d][:, off:off + w],
                            start=(d == 0), stop=(d == DT - 1))
                        off += w
                off = 0
                for c, w in enumerate(kch):
                    nc.vector.tensor_scalar_add(
                        k_t[m][:, off:off + w], pk[c][:], bk_sb[:, m:m + 1])
                    off += w

        def a1(mrange):
            for m in mrange:
                pq = psum.tile([128, 1024], F32, name=f"pq{m}", tag="ps1024")
                for d in range(DT):
                    for c in range(2):
                        nc.tensor.matmul(pq[:, ts(c, 512)],
                                         wq[d][:, ts(m, 128)],
                                         xq[d][:, ts(c, 512)],
                                         start=(d == 0), stop=(d == DT - 1))
                nc.vector.tensor_scalar_add(q_t[m][:], pq[:], bq_sb[:, m:m + 1])

        def b_pairs(qc, prange):
            for p in prange:
                h0, h1 = 2 * p, 2 * p + 1
                o0 = psum.tile([128, 512], F32, name=f"o0_{qc}_{p}", tag="oB",
                               bufs=2)
                o1 = psum.tile([128, 512], F32, name=f"o1_{qc}_{p}", tag="oB",
                               bufs=2)
                for kt in range(KCT):
                    s = psum.tile([128, 1024], F32,
                                  name=f"s_{qc}_{p}_{kt}", tag="ps1024")
                    nc.tensor.matmul(
                        s[:, 0:512], k_t[p][0:64, ts(kt, 128)],
                        q_t[p][0:64, ts(qc, 512)])
                    nc.tensor.matmul(
                        s[:, 512:1024], k_t[p][64:128, ts(kt, 128)],
                        q_t[p][64:128, ts(qc, 512)])
                    pp = ppool.tile([128, 1024], BF16,
                                    name=f"pp_{qc}_{p}_{kt}", tag="pp")
                    nc.scalar.activation(
                        pp[:], s[:], AF.Exp,
                        bias=mb_sb[:, kt:kt + 1], scale=float(SCALE))
                    nc.tensor.matmul(
                        o0[0:65, :], v65[kt][:, h0 * 65:(h0 + 1) * 65],
                        pp[:, 0:512],
                        start=(kt == 0), stop=(kt == KCT - 1))
                    nc.tensor.matmul(
                        o1[0:65, :], v65[kt][:, h1 * 65:(h1 + 1) * 65],
                        pp[:, 512:1024],
                        start=(kt == 0), stop=(kt == KCT - 1))
                nc.vector.tensor_copy(o_t[p][0:64, ts(qc, 512)], o0[0:64, :])
                nc.vector.tensor_copy(o_t[p][64:128, ts(qc, 512)], o1[0:64, :])
                # stage sums rows (head h -> tile h%4, row 32*(h//4);
                # engine partition starts must be 32-aligned)
                for h, ops in ((h0, o0), (h1, o1)):
                    g, r = h % 4, 32 * (h // 4)
                    nc.vector.tensor_copy(stg[g][r:r + 1, :], ops[64:65, :])

        def b_denoms(qc):
            # exact reciprocals, 4 lanes-parallel [128,512] calls, then
            # DRAM bounce + 0-stride broadcast + in-place normalize
            rstg = [rcpool.tile([128, 512], F32,
                                name=f"rstg_{qc}_{g}", tag=f"rstg{g}")
                    for g in range(4)]
            for g in range(4):
                nc.vector.reciprocal(rstg[g][:], stg[g][:])
                for hh in range(4):
                    h = hh * 4 + g
                    nc.sync.dma_start(
                        rdr[qc][h:h + 1, :], rstg[g][32 * hh:32 * hh + 1, :])
            for p in range(NP):
                bcs = bcpool.tile([128, 1024], F32,
                                  name=f"bcs_{qc}_{p}", tag="bcs")
                nc.sync.dma_start(
                    bcs[0:64, 0:512],
                    rdr[qc][2 * p:2 * p + 1, :].to_broadcast([64, 512]))
                nc.sync.dma_start(
                    bcs[64:128, 512:1024],
                    rdr[qc][2 * p + 1:2 * p + 2, :].to_broadcast([64, 512]))
                nc.vector.tensor_mul(
                    o_t[p][0:64, ts(qc, 512)],
                    o_t[p][0:64, ts(qc, 512)], bcs[0:64, 0:512])
                nc.vector.tensor_mul(
                    o_t[p][64:128, ts(qc, 512)],
                    o_t[p][64:128, ts(qc, 512)], bcs[64:128, 512:1024])

        def c_fc(qtrange):
            for qt in qtrange:
                fp = psum.tile([128, 1024], F32, name=f"fp{qt}", tag="ps1024")
                for dt in range(DT):
                    for c in range(2):
                        nc.tensor.matmul(fp[:, ts(c, 512)],
                                         o_t[dt][:, ts(qt, 128)],
                                         wf[dt][:, ts(c, 512)],
                                         start=(dt == 0), stop=False)
                for c in range(2):
                    nc.tensor.matmul(fp[:, ts(c, 512)], ones128[0:1, 0:128],
                                     bf_sb[0:1, ts(c, 512)],
                                     start=False, stop=True)
                ost = outpool.tile([128, 1024], F32, name=f"ost{qt}", tag="ost")
                nc.vector.tensor_copy(ost[:], fp[:])
                nc.sync.dma_start(out_d[ts(qt, 128), :], ost[:])

        # ---------------- emission order ----------------
        a3(range(KCT))
        for d in range(DT):
            nc.sync.dma_start(wk[d][:], wk_d[ts(d, 128), :])
        a2(range(0, 4))
        for d in range(DT):
            nc.sync.dma_start(xq[d][:], xq_d[ts(d, 128), :])
            nc.sync.dma_start(wq[d][:], wq_d[ts(d, 128), :])
        a1(range(0, 4))
        b_pairs(0, range(0, 4))
        a2(range(4, 8))
        a1(range(4, 8))
        b_pairs(0, range(4, 8))
        b_denoms(0)
        for d in range(DT):
            nc.sync.dma_start(wf[d][:], wf_d[ts(d, 128), :])
        b_pairs(1, range(0, 4))
        c_fc(range(0, 2))
        b_pairs(1, range(4, 8))
        c_fc(range(2, 4))
        b_denoms(1)
        c_fc(range(4, 8))

    nc.finalize()
    return nc


_LDW_PATCHED = False


def _enable_ldw_opt():
    global _LDW_PATCHED
    if _LDW_PATCHED:
        return
    import concourse.bass_utils as bu
    orig = bu.run_command

    def patched(cmd, *a, **k):
        cmd = [c.replace("--enable-ldw-opt=false", "--enable-ldw-opt=true")
               if isinstance(c, str) else c for c in cmd]
        return orig(cmd, *a, **k)

    bu.run_command = patched
    _LDW_PATCHED = True


def _get_program(KC):
    if KC not in _programs:
        if __import__("os").environ.get("LDW_OPT"):
            _enable_ldw_opt()
        _programs[KC] = _build(KC)
    return _programs[KC]


LAST_EXEC_NS = None
PROFILE = False


def _ensure_profile_hook():
    """Wire up the NTFF profile hook that the slim agent container leaves
    unconnected (antenv.axon_hooks is not injected; the ctypes hook body
    ships in trn_agent_boot)."""
    import types

    try:
        from antenv.axon_hooks import get_axon_ntff_profile_hook  # noqa: F401
        return
    except ImportError:
        pass
    import antenv

    mod = types.ModuleType("antenv.axon_hooks")
    _h = [None]
    mod.set_axon_ntff_profile_hook = lambda h: _h.__setitem__(0, h)
    mod.get_axon_ntff_profile_hook = lambda: _h[0]
    sys.modules["antenv.axon_hooks"] = mod
    antenv.axon_hooks = mod
    from trn_agent_boot.trn_boot import _ntff_profile_via_ctypes

    mod.set_axon_ntff_profile_hook(
        _ntff_profile_via_ctypes("/opt/axon/libaxon_pjrt.so"))
    # artifact upload needs a bucket this container doesn't have
    import concourse.bass_utils as bu

    bu.upload_artifacts = lambda tmpdir: f"local:{tmpdir}"


def kernel(x, mask, Wq, bq, Wk, bk, Wv, bv, Wf, bf):
    global LAST_EXEC_NS
    from concourse.bass_utils import run_bass_kernel_spmd

    if PROFILE:
        _ensure_profile_hook()

    x = np.asarray(x, dtype=np.float32)
    mask = np.asarray(mask)
    Wq16, Wk16, Wv16, Wf16 = (
        np.ascontiguousarray(np.asarray(w).astype(BF16NP))
        for w in (Wq, Wk, Wv, Wf))
    bq, bk = (np.asarray(v, np.float32) for v in (bq, bk))
    bv16, bf16v = (np.asarray(v).astype(BF16NP).reshape(1, D)
                   for v in (bv, bf))

    keeps = [np.nonzero(np.asarray(mask[b]) == 0)[0] for b in range(BS)]
    maxk = max(1, max(len(k) for k in keeps))
    KC = -(-maxk // 128) * 128
    nc = _get_program(KC)
    KCT = KC // 128

    bq_t = np.ascontiguousarray(bq.reshape(MT, 128).T)
    bk_t = np.ascontiguousarray(bk.reshape(MT, 128).T)

    x16 = x.astype(BF16NP)
    in_maps = []
    for c in range(8):
        b, j = divmod(c, 2)
        keep = keeps[b]
        xq_t = np.ascontiguousarray(x16[b, j * QH:(j + 1) * QH, :].T)
        xkv_t = np.zeros((D, KC), BF16NP)
        xkv_t[:, :len(keep)] = x16[b, keep, :].T
        mbv = np.full(KC, NEG, np.float32)
        mbv[:len(keep)] = 0.0
        mb_t = np.ascontiguousarray(mbv.reshape(KCT, 128).T)
        in_maps.append({
            "ones": _ONES, "xq": xq_t, "xkv": xkv_t,
            "wq": Wq16, "wk": Wk16, "wv": Wv16, "wf": Wf16,
            "mb": mb_t, "bqt": bq_t, "bkt": bk_t,
            "bvr": bv16, "bfr": bf16v,
        })

    res = run_bass_kernel_spmd(nc, in_maps, core_ids=list(range(8)),
                               trace=PROFILE)
    if res.exec_time_ns is not None:
        LAST_EXEC_NS = res.exec_time_ns

    out = np.empty((BS, L, D), np.float32)
    for c in range(8):
        b, j = divmod(c, 2)
        out[b, j * QH:(j + 1) * QH, :] = res.results[c]["out"]
    return out


# revision 20
# speedup vs baseline: 1.0072x; 1.0056x over previous
"""Trainium2 Bass kernel for nn_MultiHeadAttention (BS=4, L=2048, D=1024, H=16).

Sharding: 8 cores = 4 batches x 2 query-halves. Each core computes attention
for 1024 query rows of one batch over all 16 heads, with K/V restricted to
that batch's unmasked key positions (host-side gather; masked keys contribute
exactly 0 to the reference softmax, and padding slots get bias -1e30 so
exp() makes them exactly 0 on device too). No cross-core communication; the
host concatenates the 8 [1024, 1024] outputs.

Per-core dataflow (everything kept transposed / d-major so no on-device
transposes are ever needed):
  A3: V[tok, dv]   = (xkv^T)^T @ Wv + bv      (stationary = xkv^T tiles)
      stored with a ones-column appended per head (65 cols/head) so that the
      PV matmul also produces the softmax denominator in its 65th row.
  A2: K^T[dv, tok] = Wk^T-tiles @ xkv^T + bk
  A1: Q^T[dv, tok] = Wq^T-tiles @ xq^T + bq
  B:  per (query-chunk qc of 512, head-pair p):
        S^T[k,q] via row-packed K=64 matmul pairs (2 heads concurrently on
        the PE via partition offsets 0/64) -> 2-bank PSUM [128, 1024]
        exp via one ScalarE ACTIVATE (scale=1/8, bias = mask column) -> P^T
        O^T[0:64] + sums[64] accumulate via M=65 PV matmuls; sums rows are
        staged 32-aligned, inverted with 4 lane-parallel exact reciprocals
        per chunk, broadcast across partitions with 0-stride DMAs from a
        DRAM bounce, and multiplied in-place into o_t.
  C:  out[q, n] = o_t-tiles^T @ Wf + bf, streamed out per 128-row tile.

Matmul operands are bf16 with N=1024 moving where PSUM allows (fp32r lowers
to the 2x-slower fp32-HIGH path on this toolchain); accumulation is fp32 in
PSUM. Emission order interleaves the projection tail and fc with the
ACT-bound attention phase so the Tile scheduler can fill PE gaps.
"""
import sys

sys.path.insert(0, "/opt/trn_rl_repo")

import numpy as np
import ml_dtypes

BF16NP = ml_dtypes.bfloat16

BS, L, D, H, DK = 4, 2048, 1024, 16, 64
SCALE = 1.0 / np.sqrt(DK)
QH = L // 2          # queries per core
NP = H // 2          # head pairs
MT = D // 128        # out-dim tiles
DT = D // 128        # contraction d tiles
NEG = -1.0e30

_programs = {}
_ONES = np.ones((128, 128), BF16NP)


def _chunks(total, maxc=512):
    n = -(-total // maxc)
    base = total // n
    rem = total - base * n
    return [base + (1 if i < rem else 0) for i in range(n)]


def _build(KC, with_bias=True):
    import contextlib

    import concourse.bacc as bacc
    import concourse.bass as bass
    import concourse.tile as tile
    import concourse.mybir as mybir

    F32 = mybir.dt.float32
    BF16 = mybir.dt.bfloat16
    AF = mybir.ActivationFunctionType
    ts = bass.ts

    KCT = KC // 128
    kch = _chunks(KC)

    nc = bacc.Bacc("TRN2", target_bir_lowering=False)

    xq_d = nc.dram_tensor("xq", [D, QH], BF16, kind="ExternalInput")
    xkv_d = nc.dram_tensor("xkv", [D, KC], BF16, kind="ExternalInput")
    wq_d = nc.dram_tensor("wq", [D, D], BF16, kind="ExternalInput")
    wk_d = nc.dram_tensor("wk", [D, D], BF16, kind="ExternalInput")
    wv_d = nc.dram_tensor("wv", [D, D], BF16, kind="ExternalInput")
    wf_d = nc.dram_tensor("wf", [D, D], BF16, kind="ExternalInput")
    mb_d = nc.dram_tensor("mb", [128, KCT], F32, kind="ExternalInput")
    ones_d = nc.dram_tensor("ones", [128, 128], BF16, kind="ExternalInput")
    bq_d = nc.dram_tensor("bqt", [128, MT], F32, kind="ExternalInput")
    bk_d = nc.dram_tensor("bkt", [128, MT], F32, kind="ExternalInput")
    bv_d = nc.dram_tensor("bvr", [1, D], BF16, kind="ExternalInput")
    bf_d = nc.dram_tensor("bfr", [1, D], BF16, kind="ExternalInput")
    out_d = nc.dram_tensor("out", [QH, D], F32, kind="ExternalOutput")

    with tile.TileContext(nc) as tc, nc.allow_low_precision(
        reason="bf16 matmul pipeline with fp32 accumulation"
    ), contextlib.ExitStack() as ctx:
        const = ctx.enter_context(tc.tile_pool(name="const", bufs=1))
        persist = ctx.enter_context(tc.tile_pool(name="persist", bufs=1))
        work = ctx.enter_context(tc.tile_pool(name="work", bufs=1))
        ppool = ctx.enter_context(tc.tile_pool(name="ppool", bufs=3))
        rcpool = ctx.enter_context(tc.tile_pool(name="rcpool", bufs=1))
        bcpool = ctx.enter_context(tc.tile_pool(name="bcpool", bufs=2))
        outpool = ctx.enter_context(tc.tile_pool(name="outpool", bufs=1))
        psum = ctx.enter_context(tc.tile_pool(name="psum", bufs=3, space="PSUM"))
        drpool = ctx.enter_context(tc.tile_pool(name="drpool", bufs=1, space="DRAM"))

        ones128 = const.tile([128, 128], BF16, name="ones128")
        nc.sync.dma_start(ones128[:], ones_d[:])
        mb_sb = const.tile([128, KCT], F32, name="mb_sb")
        nc.sync.dma_start(mb_sb[:], mb_d[:])
        bq_sb = const.tile([128, MT], F32, name="bq_sb")
        nc.sync.dma_start(bq_sb[:], bq_d[:])
        bk_sb = const.tile([128, MT], F32, name="bk_sb")
        nc.sync.dma_start(bk_sb[:], bk_d[:])
        bv_sb = const.tile([1, D], BF16, name="bv_sb")
        nc.sync.dma_start(bv_sb[:], bv_d[:])
        bf_sb = const.tile([1, D], BF16, name="bf_sb")
        nc.sync.dma_start(bf_sb[:], bf_d[:])

        q_t = [persist.tile([128, QH], BF16, name=f"q_t{m}", tag=f"q_t{m}")
               for m in range(MT)]
        k_t = [persist.tile([128, KC], BF16, name=f"k_t{m}", tag=f"k_t{m}")
               for m in range(MT)]
        v65 = [persist.tile([128, H * 65], BF16, name=f"v65_{t}", tag=f"v65_{t}")
               for t in range(KCT)]
        o_t = [persist.tile([128, QH], BF16, name=f"o_t{p}", tag=f"o_t{p}")
               for p in range(NP)]
        xkv = [persist.tile([128, KC], BF16, name=f"xkv{d}", tag=f"xkv{d}")
               for d in range(DT)]
        xq = [persist.tile([128, QH], BF16, name=f"xq{d}", tag=f"xq{d}")
              for d in range(DT)]
        wv = [persist.tile([128, D], BF16, name=f"wv{d}", tag=f"wv{d}")
              for d in range(DT)]
        wk = [persist.tile([128, D], BF16, name=f"wk{d}", tag=f"wk{d}")
              for d in range(DT)]
        wq = [persist.tile([128, D], BF16, name=f"wq{d}", tag=f"wq{d}")
              for d in range(DT)]
        wf = [persist.tile([128, D], BF16, name=f"wf{d}", tag=f"wf{d}")
              for d in range(DT)]
        stg = [work.tile([128, 512], F32, name=f"stg{g}", tag=f"stg{g}")
               for g in range(4)]
        for g in range(4):
            nc.vector.memset(stg[g][:], 1.0)
        rdr = [drpool.tile([H, 512], F32, name=f"rdr{qc}", tag=f"rdr{qc}")
               for qc in range(2)]

        for d in range(DT):
            nc.sync.dma_start(xkv[d][:], xkv_d[ts(d, 128), :])
            nc.sync.dma_start(wv[d][:], wv_d[ts(d, 128), :])
        for t in range(KCT):
            v3 = v65[t].rearrange("p (h e) -> p h e", e=65)
            nc.vector.tensor_copy(
                v3[:, :, 64:65],
                ones128[:, 0:H].rearrange("p (h e) -> p h e", e=1))

        # ---------------- phase pieces ----------------
        def a3(trange):
            # V (k-major) with ones column interleaved; N=1024 moving
            for t in trange:
                pv = psum.tile([128, 1024], F32, name=f"pv{t}", tag="ps1024")
                for d in range(DT):
                    for c in range(2):
                        nc.tensor.matmul(pv[:, ts(c, 512)],
                                         xkv[d][:, ts(t, 128)],
                                         wv[d][:, ts(c, 512)],
                                         start=(d == 0),
                                         stop=(not with_bias and d == DT - 1))
                if with_bias:
                    for c in range(2):
                        nc.tensor.matmul(pv[:, ts(c, 512)], ones128[0:1, 0:128],
                                         bv_sb[0:1, ts(c, 512)],
                                         start=False, stop=True)
                dst = v65[t].rearrange("p (h e) -> p h e", e=65)
                src = pv.rearrange("p (h e) -> p h e", e=64)
                nc.vector.tensor_copy(dst[:, :, 0:64], src[:])

        def a2(mrange):
            for m in mrange:
                pk = [psum.tile([128, w], F32, name=f"pk{m}_{c}", tag="ps1024")
                      for c, w in enumerate(kch)]
                for d in range(DT):
                    off = 0
                    for c, w in enumerate(kch):
                        nc.tensor.matmul(
                            pk[c][:], wk[d][:, ts(m, 128)],
                            xkv[d][:, off:off + w],
                            start=(d == 0), stop=(d == DT - 1))
                        off += w
                off = 0
                for c, w in enumerate(kch):
                    nc.vector.tensor_scalar_add(
                        k_t[m][:, off:off + w], pk[c][:], bk_sb[:, m:m + 1])
                    off += w

        def a1(mrange):
            for m in mrange:
                pq = psum.tile([128, 1024], F32, name=f"pq{m}", tag="ps1024")
                for d in range(DT):
                    for c in range(2):
                        nc.tensor.matmul(pq[:, ts(c, 512)],
                                         wq[d][:, ts(m, 128)],
                                         xq[d][:, ts(c, 512)],
                                         start=(d == 0), stop=(d == DT - 1))
                nc.vector.tensor_scalar_add(q_t[m][:], pq[:], bq_sb[:, m:m + 1])

        def b_pairs(qc, prange):
            for p in prange:
                h0, h1 = 2 * p, 2 * p + 1
                o0 = psum.tile([128, 512], F32, name=f"o0_{qc}_{p}", tag="oB",
                               bufs=2)
                o1 = psum.tile([128, 512], F32, name=f"o1_{qc}_{p}", tag="oB",
                               bufs=2)
                for kt in range(KCT):
                    s = psum.tile([128, 1024], F32,
                                  name=f"s_{qc}_{p}_{kt}", tag="ps1024")
                    nc.tensor.matmul(
                        s[:, 0:512], k_t[p][0:64, ts(kt, 128)],
                        q_t[p][0:64, ts(qc, 512)])
                    nc.tensor.matmul(
                        s[:, 512:1024], k_t[p][64:128, ts(kt, 128)],
                        q_t[p][64:128, ts(qc, 512)])
                    pp = ppool.tile([128, 1024], BF16,
                                    name=f"pp_{qc}_{p}_{kt}", tag="pp")
                    nc.scalar.activation(
                        pp[:], s[:], AF.Exp,
                        bias=mb_sb[:, kt:kt + 1], scale=float(SCALE))
                    nc.tensor.matmul(
                        o0[0:65, :], v65[kt][:, h0 * 65:(h0 + 1) * 65],
                        pp[:, 0:512],
                        start=(kt == 0), stop=(kt == KCT - 1))
                    nc.tensor.matmul(
                        o1[0:65, :], v65[kt][:, h1 * 65:(h1 + 1) * 65],
                        pp[:, 512:1024],
                        start=(kt == 0), stop=(kt == KCT - 1))
                nc.vector.tensor_copy(o_t[p][0:64, ts(qc, 512)], o0[0:64, :])
                nc.vector.tensor_copy(o_t[p][64:128, ts(qc, 512)], o1[0:64, :])
                # stage sums rows (head h -> tile h%4, row 32*(h//4);
                # engine partition starts must be 32-aligned)
                for h, ops in ((h0, o0), (h1, o1)):
                    g, r = h % 4, 32 * (h // 4)
                    nc.vector.tensor_copy(stg[g][r:r + 1, :], ops[64:65, :])

        def b_denoms(qc):
            # exact reciprocals, 4 lanes-parallel [128,512] calls, then
            # DRAM bounce + 0-stride broadcast + in-place normalize
            rstg = [rcpool.tile([128, 512], F32,
                                name=f"rstg_{qc}_{g}", tag=f"rstg{g}")
                    for g in range(4)]
            for g in range(4):
                nc.vector.reciprocal(rstg[g][:], stg[g][:])
                for hh in range(4):
                    h = hh * 4 + g
                    nc.sync.dma_start(
                        rdr[qc][h:h + 1, :], rstg[g][32 * hh:32 * hh + 1, :])
            for p in range(NP):
                bcs = bcpool.tile([128, 1024], F32,
                                  name=f"bcs_{qc}_{p}", tag="bcs")
                nc.sync.dma_start(
                    bcs[0:64, 0:512],
                    rdr[qc][2 * p:2 * p + 1, :].to_broadcast([64, 512]))
                nc.sync.dma_start(
                    bcs[64:128, 512:1024],
                    rdr[qc][2 * p + 1:2 * p + 2, :].to_broadcast([64, 512]))
                nc.vector.tensor_mul(
                    o_t[p][0:64, ts(qc, 512)],
                    o_t[p][0:64, ts(qc, 512)], bcs[0:64, 0:512])
                nc.vector.tensor_mul(
                    o_t[p][64:128, ts(qc, 512)],
                    o_t[p][64:128, ts(qc, 512)], bcs[64:128, 512:1024])

        def c_fc(qtrange):
            for qt in qtrange:
                fp = psum.tile([128, 1024], F32, name=f"fp{qt}", tag="ps1024")
                for dt in range(DT):
                    for c in range(2):
                        nc.tensor.matmul(fp[:, ts(c, 512)],
                                         o_t[dt][:, ts(qt, 128)],
                                         wf[dt][:, ts(c, 512)],
                                         start=(dt == 0),
                                         stop=(not with_bias and dt == DT - 1))
                if with_bias:
                    for c in range(2):
                        nc.tensor.matmul(fp[:, ts(c, 512)], ones128[0:1, 0:128],
                                         bf_sb[0:1, ts(c, 512)],
                                         start=False, stop=True)
                ost = outpool.tile([128, 1024], F32, name=f"ost{qt}", tag="ost")
                nc.vector.tensor_copy(ost[:], fp[:])
                nc.sync.dma_start(out_d[ts(qt, 128), :], ost[:])

        # ---------------- emission order ----------------
        a3(range(KCT))
        for d in range(DT):
            nc.sync.dma_start(wk[d][:], wk_d[ts(d, 128), :])
        a2(range(0, 4))
        for d in range(DT):
            nc.sync.dma_start(xq[d][:], xq_d[ts(d, 128), :])
            nc.sync.dma_start(wq[d][:], wq_d[ts(d, 128), :])
        a1(range(0, 4))
        b_pairs(0, range(0, 4))
        a2(range(4, 8))
        a1(range(4, 8))
        b_pairs(0, range(4, 8))
        b_denoms(0)
        for d in range(DT):
            nc.sync.dma_start(wf[d][:], wf_d[ts(d, 128), :])
        b_pairs(1, range(0, 4))
        c_fc(range(0, 2))
        b_pairs(1, range(4, 8))
        c_fc(range(2, 4))
        b_denoms(1)
        c_fc(range(4, 8))

    nc.finalize()
    return nc


_LDW_PATCHED = False


def _enable_ldw_opt():
    global _LDW_PATCHED
    if _LDW_PATCHED:
        return
    import concourse.bass_utils as bu
    orig = bu.run_command

    def patched(cmd, *a, **k):
        cmd = [c.replace("--enable-ldw-opt=false", "--enable-ldw-opt=true")
               if isinstance(c, str) else c for c in cmd]
        return orig(cmd, *a, **k)

    bu.run_command = patched
    _LDW_PATCHED = True


def _get_program(KC, with_bias=True):
    key = (KC, with_bias)
    if key not in _programs:
        if __import__("os").environ.get("LDW_OPT"):
            _enable_ldw_opt()
        _programs[key] = _build(KC, with_bias)
    return _programs[key]


LAST_EXEC_NS = None
PROFILE = False


def _ensure_profile_hook():
    """Wire up the NTFF profile hook that the slim agent container leaves
    unconnected (antenv.axon_hooks is not injected; the ctypes hook body
    ships in trn_agent_boot)."""
    import types

    try:
        from antenv.axon_hooks import get_axon_ntff_profile_hook  # noqa: F401
        return
    except ImportError:
        pass
    import antenv

    mod = types.ModuleType("antenv.axon_hooks")
    _h = [None]
    mod.set_axon_ntff_profile_hook = lambda h: _h.__setitem__(0, h)
    mod.get_axon_ntff_profile_hook = lambda: _h[0]
    sys.modules["antenv.axon_hooks"] = mod
    antenv.axon_hooks = mod
    from trn_agent_boot.trn_boot import _ntff_profile_via_ctypes

    mod.set_axon_ntff_profile_hook(
        _ntff_profile_via_ctypes("/opt/axon/libaxon_pjrt.so"))
    # artifact upload needs a bucket this container doesn't have
    import concourse.bass_utils as bu

    bu.upload_artifacts = lambda tmpdir: f"local:{tmpdir}"


def kernel(x, mask, Wq, bq, Wk, bk, Wv, bv, Wf, bf):
    global LAST_EXEC_NS
    from concourse.bass_utils import run_bass_kernel_spmd

    if PROFILE:
        _ensure_profile_hook()

    x = np.asarray(x, dtype=np.float32)
    mask = np.asarray(mask)
    Wq16, Wk16, Wv16, Wf16 = (
        np.ascontiguousarray(np.asarray(w).astype(BF16NP))
        for w in (Wq, Wk, Wv, Wf))
    bq, bk = (np.asarray(v, np.float32) for v in (bq, bk))
    bv16, bf16v = (np.asarray(v).astype(BF16NP).reshape(1, D)
                   for v in (bv, bf))

    keeps = [np.nonzero(np.asarray(mask[b]) == 0)[0] for b in range(BS)]
    maxk = max(1, max(len(k) for k in keeps))
    KC = -(-maxk // 128) * 128
    with_bias = bool(np.any(np.asarray(bv)) or np.any(np.asarray(bf)))
    nc = _get_program(KC, with_bias)
    KCT = KC // 128

    bq_t = np.ascontiguousarray(bq.reshape(MT, 128).T)
    bk_t = np.ascontiguousarray(bk.reshape(MT, 128).T)

    x16 = x.astype(BF16NP)
    in_maps = []
    for c in range(8):
        b, j = divmod(c, 2)
        keep = keeps[b]
        xq_t = np.ascontiguousarray(x16[b, j * QH:(j + 1) * QH, :].T)
        xkv_t = np.zeros((D, KC), BF16NP)
        xkv_t[:, :len(keep)] = x16[b, keep, :].T
        mbv = np.full(KC, NEG, np.float32)
        mbv[:len(keep)] = 0.0
        mb_t = np.ascontiguousarray(mbv.reshape(KCT, 128).T)
        in_maps.append({
            "ones": _ONES, "xq": xq_t, "xkv": xkv_t,
            "wq": Wq16, "wk": Wk16, "wv": Wv16, "wf": Wf16,
            "mb": mb_t, "bqt": bq_t, "bkt": bk_t,
            "bvr": bv16, "bfr": bf16v,
        })

    res = run_bass_kernel_spmd(nc, in_maps, core_ids=list(range(8)),
                               trace=PROFILE)
    if res.exec_time_ns is not None:
        LAST_EXEC_NS = res.exec_time_ns

    out = np.empty((BS, L, D), np.float32)
    for c in range(8):
        b, j = divmod(c, 2)
        out[b, j * QH:(j + 1) * QH, :] = res.results[c]["out"]
    return out
